# revision 14
# baseline (speedup 1.0000x reference)
"""Trainium2 Bass kernel for HTM spatial-pooler overlap + global top-k inhibition.

Problem (nn_HTMModel_19834158973432):
    overlap  = connections @ input_vector          # [4096] = [4096, 32768] @ [32768]
    boosted  = overlap * boosting_factors          # [4096]
    winners  = top_k(boosted, 82)                  # ties broken by lower index
    active   = one_hot(winners)                    # [4096] 0/1 mask
    returns (active, active * boosted)

Strategy (8 NeuronCores, SPMD):
  - connections / input_vector are exactly 0/1-valued, so the host re-encodes
    them losslessly as bit-packed uint16 (16 input positions per lane): the
    64 MiB/core f32 shard becomes a 2 MiB/core bit matrix.
  - Each core's overlap slice is a SWAR popcount of (pconn & vpack), column-
    split across two engines: DVE takes groups [0, GD) on u16 views, GpSimd
    (Q7) takes [GD, G) on u32 views (Q7 integer ALU is exact on u32; its u32
    masks come from SBUF const tiles because instruction immediates are f32).
    A column-pair fold turns the tail into u8 byte-counts that ONE ACT pass
    per row-block accumulates (accum_out).
  - Each core builds its local key slice
        key[c] = boosted[c] * 4096 + (4095 - c)
    (keys are distinct exact-int floats < 2^23; thresholding reproduces
    top_k's lower-index tie-breaking), PE-transposes the [128, 4] key tile to
    [4, 128] so the DRAM staging write is 4 contiguous descriptors, computes
    its LOCAL level-1 threshold-search counts (64 edges of width 131072), and
    AllGathers keys+counts (2.25 KB/rank).  A tiny warm-up AllGather fires
    mid-popcount to keep the TOPSP collectives firmware out of deep idle.
  - Every core (redundantly) finishes the branch-free 64-ary threshold
    search for the 82nd-largest key: level 1 comes free from the summed
    gathered counts; levels 2-4 run on the [128, 32] gathered-key layout
    (is_ge masks -> ones-matmul partition reduction -> prefix count).
  - boosted is reconstructed from keys as (key - negidx)/4096 (exact).
  - Each core writes the full [2, 4096] output; the host returns core 0's.
"""

import sys

if "/opt/trn_rl_repo" not in sys.path:
    sys.path.insert(0, "/opt/trn_rl_repo")

import numpy as np

C_TOT = 4096          # minicolumns
IN = 32768            # input size
CORES = 8
ROWS = C_TOT // CORES  # 512 rows per core
K_ACT = 82            # active columns per inhibition area
RB = ROWS // 128      # 4 row blocks of 128 partitions per core
G = IN // 16          # 2048 packed uint16 groups along the input axis
H = G // 2            # half-width after the column-pair fold

# column split: DVE SWARs groups [0, GD), GpSimd (Q7, u32 views) [GD, G).
# Balance point: DVE ~2.4 u16 elem/cyc/lane, Q7 ~2.6 cyc per u32 elem.
GD = 1536             # DVE's share of the 2048 u16 groups (multiple of 4)
HD = GD // 2          # DVE's share of the folded half-width
GG = G - GD           # GpSimd's share (u16 groups)

WIDTHS = [131072, 2048, 32, 1]  # 4-level 64-ary search over keys in [0, 2^23)
NEDGE = 64
WSUM = 133153
CCN = ROWS + NEDGE    # collective payload: 512 keys + 64 level-1 counts


def _build_nc(stage=4):
    from concourse import bacc, mybir, tile
    from concourse.ap import AP

    f32 = mybir.dt.float32
    u8 = mybir.dt.uint8
    u16 = mybir.dt.uint16
    u32 = mybir.dt.uint32
    bf16 = mybir.dt.bfloat16
    Alu = mybir.AluOpType

    nc = bacc.Bacc("TRN2", target_bir_lowering=False, debug=False,
                   enable_asserts=False, num_devices=CORES)

    pconn = nc.dram_tensor("pconn", [ROWS, G], u16, kind="ExternalInput")
    vpack = nc.dram_tensor("vpack", [G], u16, kind="ExternalInput")
    # boost4/neg4 are host-arranged so [128, RB] tiles load contiguously:
    # dram[p*RB + cb] is the value for local row i = cb*128 + p.
    boost4 = nc.dram_tensor("boost4", [ROWS], f32, kind="ExternalInput")
    neg4 = nc.dram_tensor("neg4", [ROWS], f32, kind="ExternalInput")
    ident = nc.dram_tensor("ident", [128, 128], f32, kind="ExternalInput")
    rampsf = nc.dram_tensor("rampsf", [4 * NEDGE], f32, kind="ExternalInput")
    negidx = nc.dram_tensor("negidx", [C_TOT], f32, kind="ExternalInput")
    out = nc.dram_tensor("out", [2, C_TOT], f32, kind="ExternalOutput")

    with tile.TileContext(nc) as tc:
        with (
            tc.tile_pool(name="const", bufs=1) as constp,
            tc.tile_pool(name="cpool", bufs=2) as cpool,
            tc.tile_pool(name="scrp", bufs=2) as scrp,
            tc.tile_pool(name="dramp", bufs=1, space="DRAM") as dramp,
            tc.tile_pool(name="vpsp", bufs=2, space="PSUM") as psp,
        ):
            # packed input vector broadcast to all partitions (512 KB DMA)
            vb = constp.tile([128, G], u16, name="vb")
            nc.sync.dma_start(vb[:], vpack.ap().partition_broadcast(128))
            pts = []
            for cb in range(RB):
                pt = cpool.tile([128, G], u16, name=f"pt_{cb}", tag="pt")
                # split per engine share so each AND starts sooner
                nc.sync.dma_start(
                    pt[:, 0:GD], pconn.ap()[cb * 128:(cb + 1) * 128, 0:GD])
                nc.sync.dma_start(
                    pt[:, GD:G], pconn.ap()[cb * 128:(cb + 1) * 128, GD:G])
                pts.append(pt)
            boost4t = constp.tile([128, RB], f32, name="boost4t")
            nc.sync.dma_start(boost4t[:],
                              boost4.ap().rearrange("(p c) -> p c", p=128))
            neg4t = constp.tile([128, RB], f32, name="neg4t")
            nc.sync.dma_start(neg4t[:],
                              neg4.ap().rearrange("(p c) -> p c", p=128))
            identt = constp.tile([128, 128], f32, name="identt")
            nc.sync.dma_start(identt[:], ident.ap())
            negidx32 = constp.tile([128, 32], f32, name="negidx32")
            nc.sync.dma_start(negidx32[:],
                              negidx.ap().rearrange("(p f) -> p f", p=128))
            ones_bf = constp.tile([128, 128], bf16, name="ones_bf")
            nc.vector.memset(ones_bf[:], 1.0)
            onesf8 = constp.tile([8, 128], f32, name="onesf8")
            nc.vector.memset(onesf8[:], 1.0)
            rampr = constp.tile([128, 4 * NEDGE], f32, name="rampr")
            nc.sync.dma_start(rampr[:], rampsf.ap().partition_broadcast(128))
            ov4 = constp.tile([128, RB], f32, name="ov4")
            actscr = constp.tile([128, H], u16, name="actscr")

            # warm-up collective plumbing (fired mid-popcount below)
            dummy_in = dramp.tile([8], f32, name="dummy_in")
            dummy_out = dramp.tile([8 * CORES], f32, name="dummy_out",
                                   addr_space="Shared")

            # ---- packed popcount matvec: 4 row blocks of 128 rows ----
            for cb in range(RB):
                pt = pts[cb]
                x = scrp.tile([128, G], u16, name=f"x_{cb}", tag="x")
                t = scrp.tile([128, G], u16, name=f"t_{cb}", tag="t")
                x1 = scrp.tile([128, G], u16, name=f"x1_{cb}", tag="x1")
                t2 = scrp.tile([128, G], u16, name=f"t2_{cb}", tag="t2")
                x1m = scrp.tile([128, G], u16, name=f"x1m_{cb}", tag="x1m")
                x2 = scrp.tile([128, G], u16, name=f"x2_{cb}", tag="x2")
                cs = scrp.tile([128, H], u16, name=f"cs_{cb}", tag="cs")
                t4 = scrp.tile([128, H], u16, name=f"t4_{cb}", tag="t4")
                csm = scrp.tile([128, H], u16, name=f"csm_{cb}", tag="csm")
                m8 = scrp.tile([128, H], u16, name=f"m8_{cb}", tag="m8")

                # --- DVE SWAR chain (Pool cannot run bitwise int ops,
                # so the whole chain stays on DVE; the AND is split at GD
                # to start on the first DMA half sooner) ---
                nc.vector.tensor_tensor(
                    x[:, 0:GD].bitcast(u32), pt[:, 0:GD].bitcast(u32),
                    vb[:, 0:GD].bitcast(u32), Alu.bitwise_and)
                nc.vector.tensor_tensor(
                    x[:, GD:G].bitcast(u32), pt[:, GD:G].bitcast(u32),
                    vb[:, GD:G].bitcast(u32), Alu.bitwise_and)
                nc.vector.tensor_scalar(
                    out=t[:], in0=x[:], scalar1=1, scalar2=0x5555,
                    op0=Alu.logical_shift_right, op1=Alu.bitwise_and)
                nc.vector.tensor_tensor(x1[:], x[:], t[:], Alu.subtract)
                nc.vector.tensor_scalar(
                    out=t2[:], in0=x1[:], scalar1=2, scalar2=0x3333,
                    op0=Alu.logical_shift_right, op1=Alu.bitwise_and)
                nc.vector.tensor_scalar(
                    out=x1m[:], in0=x1[:], scalar1=0x3333, scalar2=None,
                    op0=Alu.bitwise_and)
                nc.vector.tensor_tensor(x2[:], x1m[:], t2[:], Alu.add)
                nc.vector.tensor_tensor(cs[:], x2[:, 0:H], x2[:, H:G],
                                        Alu.add)
                nc.vector.tensor_scalar(
                    out=t4[:], in0=cs[:], scalar1=4, scalar2=0x0F0F,
                    op0=Alu.logical_shift_right, op1=Alu.bitwise_and)
                nc.vector.tensor_scalar(
                    out=csm[:], in0=cs[:], scalar1=0x0F0F, scalar2=None,
                    op0=Alu.bitwise_and)
                nc.vector.tensor_tensor(m8[:], csm[:], t4[:], Alu.add)

                # one ACT pass accumulates all 2G u8 byte-counts of the row
                nc.scalar.activation(actscr[:].bitcast(u8), m8[:].bitcast(u8),
                                     mybir.ActivationFunctionType.Copy,
                                     accum_out=ov4[:, cb:cb + 1])

                if cb == 1:
                    # warm-up: tiny AllGather keeps ncfw polling so the real
                    # collective's doorbell is serviced quickly
                    nc.sync.dma_start(
                        dummy_in.rearrange("(p f) -> p f", p=1),
                        negidx32[0:1, 0:8])
                    nc.gpsimd.collective_compute(
                        "AllGather", Alu.bypass,
                        replica_groups=[list(range(CORES))],
                        ins=[dummy_in.opt()],
                        outs=[dummy_out.opt()],
                    )

            # ---- local keys: key = overlap*boost*4096 + (4095 - c) ----
            # (boost4t is host-prescaled by 4096)
            key4 = constp.tile([128, RB], f32, name="key4")
            nc.vector.tensor_tensor(key4[:], ov4[:], boost4t[:], Alu.mult)
            nc.vector.tensor_tensor(key4[:], key4[:], neg4t[:], Alu.add)

            if stage <= 1:
                nc.sync.dma_start(
                    out.ap()[0][0:ROWS].rearrange("(c p) -> p c", p=128),
                    key4[:])
                nc.sync.dma_start(
                    out.ap()[1][0:ROWS].rearrange("(c p) -> p c", p=128),
                    ov4[:])
            if stage >= 2:
                # transpose keys to [4, 128] so the DRAM staging write is
                # 4 contiguous descriptors in local-row order i = cb*128+p.
                keyT = psp.tile([RB, 128], f32, name="keyT")
                nc.tensor.transpose(keyT[:], key4[:], identt[:])
                keyTs = constp.tile([RB, 128], f32, name="keyTs")
                nc.vector.tensor_copy(keyTs[:], keyT[:])

                # local level-1 counts: lcnt[e] = #{local keys >= e*131072}
                key4_bc = AP(key4[:].tensor, key4[:].offset,
                             [key4[:].ap[0], [1, RB], [0, NEDGE]])
                ramp0_bc = AP(rampr[:].tensor, rampr[:].offset,
                              [rampr[:].ap[0], [0, RB], [1, NEDGE]])
                mask_l1 = constp.tile([128, RB, NEDGE], bf16, name="mask_l1")
                nc.vector.tensor_tensor(mask_l1[:], key4_bc, ramp0_bc,
                                        Alu.is_ge)
                l1ps = psp.tile([128, RB, NEDGE], f32, name="l1ps")
                nc.tensor.matmul(l1ps[:].opt(), lhsT=ones_bf[:],
                                 rhs=mask_l1[:].opt(), start=True, stop=True)
                l1_t = AP(l1ps[:].tensor, l1ps[:].offset,
                          [l1ps[:].ap[0], [1, NEDGE], [NEDGE, RB]])
                lcnt = constp.tile([128, NEDGE], f32, name="lcnt")
                nc.vector.reduce_sum(lcnt[:], l1_t, axis=mybir.AxisListType.X)

                cc_in = dramp.tile([CCN], f32, name="cc_in")
                cc_out = dramp.tile([CORES * CCN], f32, name="cc_out",
                                    addr_space="Shared")
                nc.sync.dma_start(
                    cc_in[0:ROWS].rearrange("(p f) -> p f", p=RB),
                    keyTs[:])
                nc.sync.dma_start(
                    cc_in[ROWS:CCN].rearrange("(p f) -> p f", p=1),
                    lcnt[0:1, :])
                nc.gpsimd.collective_compute(
                    "AllGather", Alu.bypass,
                    replica_groups=[list(range(CORES))],
                    ins=[cc_in.opt()],
                    outs=[cc_out.opt()],
                )
                # gathered keys on the [128, 32] layout (c = p*32 + f)
                key32 = constp.tile([128, 32], f32, name="key32")
                for r in range(CORES):
                    nc.sync.dma_start(
                        key32[16 * r:16 * (r + 1), :],
                        cc_out[CCN * r:CCN * r + ROWS].rearrange(
                            "(b f) -> b f", b=16))
                # gathered per-rank level-1 counts -> [8, 64]
                gcnt8 = constp.tile([8, NEDGE], f32, name="gcnt8")
                gcnt_src = AP(cc_out.tensor, cc_out.offset + ROWS,
                              [[CCN, 8], [1, NEDGE]])
                nc.sync.dma_start(gcnt8[:], gcnt_src)
                # boosted = (key - (4095-c)) / 4096, exact
                boosted32 = constp.tile([128, 32], f32, name="boosted32")
                nc.vector.tensor_tensor(boosted32[:], key32[:], negidx32[:],
                                        Alu.subtract)
                nc.vector.tensor_scalar(
                    out=boosted32[:], in0=boosted32[:],
                    scalar1=1.0 / 4096.0, scalar2=None, op0=Alu.mult)

            if stage == 2:
                nc.sync.dma_start(
                    out.ap()[0].rearrange("(p f) -> p f", p=128), key32[:])
                nc.sync.dma_start(
                    out.ap()[1].rearrange("(p f) -> p f", p=128),
                    boosted32[:])

            if stage >= 3:
                # ---- finish the 4-level 64-ary threshold search ----
                # level 1 total counts = sum of the 8 gathered local counts
                l1tot = psp.tile([128, NEDGE], f32, name="l1tot")
                nc.tensor.matmul(l1tot[:].opt(), lhsT=onesf8[:],
                                 rhs=gcnt8[:].opt(), start=True, stop=True)
                sel0 = constp.tile([128, NEDGE], f32, name="sel0")
                cnt0 = constp.tile([128, 1], f32, name="cnt0")
                nc.vector.tensor_scalar(
                    out=sel0[:], in0=l1tot[:], scalar1=float(K_ACT),
                    scalar2=None, op0=Alu.is_ge, op1=Alu.add,
                    accum_out=cnt0[:])
                acur = constp.tile([128, 1], f32, name="a0")
                nc.vector.tensor_scalar(
                    out=acur[:], in0=cnt0[:], scalar1=float(WIDTHS[0]),
                    scalar2=None, op0=Alu.mult)

                for li in (1, 2, 3):
                    w = WIDTHS[li]
                    e2 = constp.tile([128, NEDGE], f32, name=f"edges{li}")
                    nc.vector.tensor_scalar(
                        out=e2[:], in0=rampr[:, li * NEDGE:(li + 1) * NEDGE],
                        scalar1=acur[:], scalar2=None, op0=Alu.add)
                    edges = e2[:]
                    edges_bc = AP(edges.tensor, edges.offset,
                                  [edges.ap[0], [0, 16], [1, NEDGE]])
                    key_bc_a = AP(key32[:].tensor, key32[:].offset,
                                  [key32[:].ap[0], [1, 16], [0, NEDGE]])
                    key_bc_b = AP(key32[:].tensor, key32[:].offset + 16,
                                  [key32[:].ap[0], [1, 16], [0, NEDGE]])
                    mask_a = scrp.tile([128, 16, NEDGE], bf16,
                                       name=f"maska{li}", tag="maska", bufs=1)
                    nc.vector.tensor_tensor(mask_a[:], key_bc_a, edges_bc,
                                            Alu.is_ge)
                    mask_b = scrp.tile([128, 16, NEDGE], bf16,
                                       name=f"maskb{li}", tag="maskb", bufs=1)
                    nc.vector.tensor_tensor(mask_b[:], key_bc_b, edges_bc,
                                            Alu.is_ge)
                    cnt_ps = psp.tile([128, 8, NEDGE], f32, name=f"cnt{li}",
                                      tag="vps")
                    for g in range(4):
                        mk = mask_a if g < 2 else mask_b
                        nc.tensor.matmul(
                            cnt_ps[:].opt(), lhsT=ones_bf[:],
                            rhs=mk[:, 8 * (g % 2):8 * (g % 2 + 1), :].opt(),
                            start=(g == 0), stop=(g == 3))
                    cnt_t = AP(cnt_ps[:].tensor, cnt_ps[:].offset,
                               [cnt_ps[:].ap[0], [1, NEDGE], [NEDGE, 8]])
                    tot = constp.tile([128, NEDGE], f32, name=f"tot{li}")
                    nc.vector.reduce_sum(tot[:], cnt_t,
                                         axis=mybir.AxisListType.X)
                    selscr = constp.tile([128, NEDGE], f32, name=f"sel{li}")
                    cnt = constp.tile([128, 1], f32, name=f"cntv{li}")
                    nc.vector.tensor_scalar(
                        out=selscr[:], in0=tot[:], scalar1=float(K_ACT),
                        scalar2=None, op0=Alu.is_ge, op1=Alu.add,
                        accum_out=cnt[:])
                    anew = constp.tile([128, 1], f32, name=f"a{li}")
                    nc.vector.tensor_scalar(
                        out=anew[:], in0=cnt[:], scalar1=float(w),
                        scalar2=acur[:], op0=Alu.mult, op1=Alu.add)
                    acur = anew

                tthr = constp.tile([128, 1], f32, name="tthr")
                nc.vector.tensor_scalar(
                    out=tthr[:], in0=acur[:], scalar1=float(-WSUM),
                    scalar2=None, op0=Alu.add)

                # ---- apply threshold, write outputs (contiguous) ----
                active32 = constp.tile([128, 32], f32, name="active32")
                nc.vector.tensor_scalar(
                    out=active32[:], in0=key32[:], scalar1=tthr[:],
                    scalar2=None, op0=Alu.is_ge)
                masked32 = constp.tile([128, 32], f32, name="masked32")
                nc.vector.tensor_tensor(masked32[:], active32[:],
                                        boosted32[:], Alu.mult)
                nc.sync.dma_start(
                    out.ap()[0].rearrange("(p f) -> p f", p=128), active32[:])
                nc.sync.dma_start(
                    out.ap()[1].rearrange("(p f) -> p f", p=128), masked32[:])

    nc.compile()
    return nc


def _pack_bits_u16(a):
    """[..., N] 0/1 f32 -> [..., N/16] uint16, bit t of group g = a[16g+t]."""
    b = np.packbits(a.astype(np.uint8), axis=-1, bitorder="little")
    return b.view("<u2").reshape(*a.shape[:-1], a.shape[-1] // 16)


def _make_in_maps(input_vector, connections, boosting_factors):
    v = np.ascontiguousarray(np.asarray(input_vector, dtype=np.float32))
    c = np.asarray(connections, dtype=np.float32)
    b = np.ascontiguousarray(np.asarray(boosting_factors, dtype=np.float32))
    vp = np.ascontiguousarray(_pack_bits_u16(v))
    neg = (float(C_TOT - 1) - np.arange(C_TOT, dtype=np.float32))
    identm = np.eye(128, dtype=np.float32)
    # per-level edge ramps (along free axis) with cumulative -w folded in
    rampsf = np.zeros((4, NEDGE), dtype=np.float32)
    csum = 0.0
    for li, w in enumerate(WIDTHS):
        rampsf[li] = np.arange(NEDGE, dtype=np.float32) * w - csum
        csum += w
    # [p, cb] tile layout: dram[p*RB + cb] holds local row i = cb*128 + p
    p_idx, cb_idx = np.divmod(np.arange(ROWS), RB)
    tile_perm = cb_idx * 128 + p_idx  # dram pos j -> local row i
    maps = []
    for r in range(CORES):
        sh = np.ascontiguousarray(
            _pack_bits_u16(c[r * ROWS:(r + 1) * ROWS]))
        bs = b[r * ROWS:(r + 1) * ROWS] * 4096.0
        ns = neg[r * ROWS:(r + 1) * ROWS]
        maps.append({
            "pconn": sh,
            "vpack": vp,
            "boost4": np.ascontiguousarray(bs[tile_perm]),
            "neg4": np.ascontiguousarray(ns[tile_perm]),
            "ident": identm,
            "rampsf": np.ascontiguousarray(rampsf.reshape(-1)),
            "negidx": neg,
        })
    return maps


def _run(input_vector, connections, boosting_factors, trace=False, stage=4):
    from concourse import bass_utils

    nc = _build_nc(stage)
    in_maps = _make_in_maps(input_vector, connections, boosting_factors)
    res = bass_utils.run_bass_kernel_spmd(
        nc, in_maps, core_ids=list(range(CORES)), trace=trace,
    )
    out = res.results[0]["out"]
    return (np.ascontiguousarray(out[0]), np.ascontiguousarray(out[1])), res


def kernel(input_vector, connections, boosting_factors):
    (active, masked), _ = _run(input_vector, connections, boosting_factors)
    return active, masked
